# revision 19
# baseline (speedup 1.0000x reference)
"""Trainium2 Bass kernel for DensityCalculator — pad-64 compact spectrum.

The radial Hamming filter kills all |k| > 16 per axis: only 33 of 64
bins survive each transform. Spectral dims use a compact-48 index set
(33 live + 15 zero) PADDED to 64-row stacked halves (partition slices
stay 64/128-aligned); the compaction shows up in the COLUMN dims and
the AllToAll wires (384 instead of 512 cols, -25%).

Pipeline: x-slab density (separable Gaussians) -> z,y forward FFT ->
A2A to kz-slabs -> x FFT + filter + inverse x, inverse y -> A2A back ->
inverse z. Complex passes are one fused block matmul (stacked [re;im]
rhs); transposes on TensorE; strided-scatter stages fp32r, contiguous
bf16; single-DMA staging/recv per collective.
"""

import os
import sys
import numpy as np

for _p in ("/opt/trn_rl_repo", "/root/.axon_site", "/root/.axon_site/_ro/trn_rl_repo",
           "/root/.axon_site/_ro/pypackages"):
    if _p not in sys.path and os.path.isdir(_p):
        sys.path.append(_p)

import concourse.bass as bass
import concourse.tile as tile
from concourse import bacc, mybir
from concourse.bass_utils import run_bass_kernel_spmd

FP = mybir.dt.float32
FR = mybir.dt.float32r
BF = mybir.dt.bfloat16
Exp = mybir.ActivationFunctionType.Exp
Copy = mybir.ActivationFunctionType.Copy

N_CORES = 8
N = 64
A = 128
K = 6
XL = 8
D1 = XL + 2 * N
KC = 48             # compact spectral bins (33 live + 15 zero)
KL = 6              # compact kz planes per core
W = 384             # compact column width (KC*8 = KL*64)

LAST_EXEC_NS = None
LAST_RESULTS = None
LAST_BKR = None
_COMPILED = None


def _build():
    nc = bacc.Bacc("TRN2", target_bir_lowering=False, debug=False,
                   num_devices=N_CORES)

    din = nc.dram_tensor("din", [A, D1 + 2 * K], FP, kind="ExternalInput").ap()
    # cmats: [WA(128) | WB(128) | WBre(64) | H2(384) | ident(64) | Az2T(128)]
    cmats = nc.dram_tensor("cmats", [128, 896], FR, kind="ExternalInput").ap()
    out = nc.dram_tensor("out", [N, 512], FP, kind="ExternalOutput").ap()

    with tile.TileContext(nc) as tc:
        with tc.tile_pool(name="const", bufs=1) as constp, \
             tc.tile_pool(name="dram", bufs=1, space="DRAM") as dram:
            din_sb = constp.tile([A, D1 + 2 * K], FP)
            nc.sync.dma_start(din_sb[:], din[:])
            cm_sb = constp.tile([128, 896], FR)
            nc.scalar.dma_start(cm_sb[:], cmats[:])

            d2_sb = din_sb[:, 0:D1]
            bwln_sb = din_sb[:, D1:D1 + 2 * K]
            WAv = cm_sb[:, 0:128]
            WBv = cm_sb[:, 128:256]
            WBrev = cm_sb[:, 256:320]
            H2v = cm_sb[:, 320:704]                # [128, 384]
            identv = cm_sb[0:N, 704:768]
            Az2Tv = cm_sb[0:N, 768:896]
            WA_bf = constp.tile([128, 128], BF)
            WB_bf = constp.tile([128, 128], BF)
            WBre_bf = constp.tile([128, N], BF)
            Az_bf = constp.tile([N, 128], BF)
            ident_bf2 = constp.tile([128, N], BF)
            ident_fr2 = constp.tile([128, N], FR)
            nc.vector.tensor_copy(WA_bf[:], WAv)
            nc.vector.tensor_copy(WB_bf[:], WBv)
            nc.scalar.activation(WBre_bf[:], WBrev, Copy)
            nc.scalar.activation(Az_bf[:], Az2Tv, Copy)
            nc.vector.tensor_copy(ident_bf2[0:N, :], identv)
            nc.vector.tensor_copy(ident_bf2[N:128, :], identv)
            nc.vector.tensor_copy(ident_fr2[0:N, :], identv)
            nc.vector.tensor_copy(ident_fr2[N:128, :], identv)

            # ---------------- Phase 1: separable density ----------------
            acc_pool = tc.tile_pool(name="p1acc", bufs=1, space="PSUM")
            accps = acc_pool.__enter__().tile([128, 512], FP, tag="acc")
            with tc.tile_pool(name="p1sb", bufs=1) as p1sb, \
                 tc.tile_pool(name="p1rho", bufs=1, space="PSUM") as p1rho:
                E = p1sb.tile([A, K, D1], BF, tag="E")
                for k in range(K):
                    nc.scalar.activation(E[:, k, :], d2_sb, Exp,
                                         bias=bwln_sb[:, K + k:K + k + 1],
                                         scale=bwln_sb[:, k:k + 1])
                G = p1sb.tile([A, K, XL, N], BF, tag="G")
                for k in range(K):
                    nc.vector.tensor_tensor(
                        G[:, k],
                        E[:, k, 0:XL][:, :, None].broadcast_to([A, XL, N]),
                        E[:, k, XL:XL + N][:, None, :].broadcast_to([A, XL, N]),
                        op=mybir.AluOpType.mult)
                rho_ps = p1rho.tile([N, 512], FP, tag="rho")
                for k in range(K):
                    nc.tensor.matmul(rho_ps[:],
                                     lhsT=E[:, k, XL + N:XL + 2 * N],
                                     rhs=G[:, k].rearrange("p x y -> p (x y)"),
                                     start=(k == 0), stop=(k == K - 1))
                rho_sb = p1sb.tile([N, 512], BF, tag="rho_sb")
                nc.vector.tensor_copy(rho_sb[:], rho_ps[:])
                # fwd z (compact-pad): accps[(kzc_re pad | kzc_im pad), (x,y)]
                nc.tensor.matmul(accps[:], lhsT=Az_bf[:], rhs=rho_sb[:],
                                 start=True, stop=True)

            # ---------------- Phase 2: distributed FFT ----------------
            fsb_pool = tc.tile_pool(name="fft", bufs=2)
            fsb = fsb_pool.__enter__()

            def cpass(Wm, s_in, out_dt=BF, tag="cstk"):
                ps = fps.tile([128, 512], FP, tag="ps")
                nc.tensor.matmul(ps[:, 0:W], lhsT=Wm, rhs=s_in,
                                 start=True, stop=True)
                o = fsb.tile([128, 512], out_dt, tag=tag)
                nc.vector.tensor_copy(o[:, 0:W], ps[:, 0:W])
                return o

            def tstage(s_in, strided_out, dt, nb, S_live, tag):
                """transpose stage, stacked [128, nb*128] input, halves at
                bases 0/64. nb blocks [64,128] per half -> [128,64] each.
                Copies keep only S_live of the 64 input-partition values:
                strided_out=False: out[q | (2t+h)*64 + p]         (p full 64)
                strided_out=True:  out[q | i*(2nb) + (2t+h)], i<S_live."""
                o = fsb.tile([128, 512], dt, tag="tstk")
                pt_re = tps.tile([128, 4, N], dt, tag=tag + "re")
                pt_im = tps.tile([128, 4, N], dt, tag=tag + "im")
                pts = [pt_re, pt_im]
                ident = ident_bf2 if dt is BF else ident_fr2
                for half in range(2):
                    src = s_in[half * N:(half + 1) * N, :]
                    for t in range(nb):
                        blkap = src[:, t * 128:(t + 1) * 128]
                        nc.tensor.transpose(pts[half][:, t, :], blkap,
                                            ident[half * N:half * N + N, :])
                for half in range(2):
                    if strided_out:
                        dst = o[half * N:(half + 1) * N, 0:S_live * 2 * nb]
                        dstv = dst.rearrange("w (i t h) -> w t i h",
                                             i=S_live, t=nb, h=2)
                    else:
                        dst = o[half * N:(half + 1) * N, 0:nb * 2 * N]
                        dstv = dst.rearrange("w (t h i) -> w t h i",
                                             t=nb, h=2, i=N)
                    ptv = pts[half]
                    for h in range(2):
                        if strided_out:
                            s_ap = ptv[h * N:(h + 1) * N, 0:nb, 0:S_live]
                            d_ap = dstv[:, :, :, h]
                        else:
                            s_ap = ptv[h * N:(h + 1) * N, 0:nb, :]
                            d_ap = dstv[:, :, h]
                        if half == 0:
                            nc.vector.tensor_copy(d_ap, s_ap)
                        else:
                            nc.scalar.activation(d_ap, s_ap, Copy)
                return o

            def unpermute(raw, a, b, tag):
                """[p | s*(a*b) + i*b + j] -> [p | i*(8b) + s*b + j]"""
                g = fsb.tile([128, 512], BF, tag=tag)
                dstv = g[:, 0:8 * a * b].rearrange(
                    "p (i s j) -> p s i j", i=a, s=8, j=b)
                srcv = raw[:, 0:8 * a * b].rearrange(
                    "p (s i j) -> p s i j", s=8, i=a, j=b)
                nc.vector.tensor_copy(dstv[0:N], srcv[0:N])
                nc.scalar.activation(dstv[N:128], srcv[N:128], Copy)
                return g

            f1 = fsb.tile([128, 512], FR, tag="f1")
            nc.vector.tensor_copy(f1[:], accps[:])
            acc_pool.__exit__(None, None, None)

            with tc.tile_pool(name="fps", bufs=1, space="PSUM") as fps, \
                 tc.tile_pool(name="ps6p", bufs=1, space="PSUM") as ps6p, \
                 tc.tile_pool(name="tps", bufs=1, space="PSUM") as tps:
                # [kz±pad | xl*64+y] -> [y± | kzc*8+xl (384)]
                t1 = tstage(f1, True, FR, 4, KC, "pf")
                # fwd y: [ky±pad | kzc*8+xl]
                f2s = cpass(WAv, t1[:, 0:W])

                # A2A #1: chunk d = cols [48d,48d+48) -> [ky±pad | kl*64+x]
                a_in = dram.tile([N_CORES, 2, N, KL, 8], BF, tag="a2a_in")
                a_out = dram.tile([N_CORES, 2, N, KL, 8], BF, tag="a2a_out")
                nc.sync.dma_start(
                    a_in.rearrange("d q p kl xl -> (q p) d (kl xl)"),
                    f2s[:, 0:W].rearrange("p (d c) -> p d c", d=8))
                nc.gpsimd.collective_compute(
                    "AllToAll", mybir.AluOpType.bypass,
                    replica_groups=[list(range(N_CORES))],
                    ins=[a_in.opt()], outs=[a_out.opt()])
                g_raw = fsb.tile([128, 512], BF, tag="graw")
                nc.sync.dma_start(
                    g_raw[:, 0:W].rearrange("p (s c) -> p s c", s=8),
                    a_out.rearrange("s q p kl xl -> (q p) s (kl xl)"))
                g = unpermute(g_raw, KL, 8, "g")    # [ky±pad | kl*64+s*8+xl]

                # -> [x± | kl*64 + ky-pad (384)]
                t2 = tstage(g, False, BF, 3, N, "p2")
                ps3 = fps.tile([128, 512], FP, tag="ps")
                nc.tensor.matmul(ps3[:, 0:W], lhsT=WA_bf[:],
                                 rhs=t2[:, 0:W], start=True, stop=True)
                f3 = fsb.tile([128, 512], BF, tag="f3")
                nc.vector.tensor_tensor(f3[:, 0:W], ps3[:, 0:W], H2v,
                                        op=mybir.AluOpType.mult)
                g4 = cpass(WB_bf[:], f3[:, 0:W], out_dt=FR)
                # [x± | kl*64+ky-pad] -> [ky±pad | x*6+kl (384)]
                t3 = tstage(g4, True, FR, 3, N, "pf")
                f5s = cpass(WBv, t3[:, 0:W])        # [y± | x*6+kl]

                # A2A #2: chunk d = cols [48d,48d+48) -> [y± | xl*48+kzc]
                a2_in = dram.tile([N_CORES, 2, N, XL, KL], BF, tag="a2a2_in")
                a2_out = dram.tile([N_CORES, 2, N, XL, KL], BF, tag="a2a2_out")
                nc.sync.dma_start(
                    a2_in.rearrange("d q p xl kl -> (q p) d (xl kl)"),
                    f5s[:, 0:W].rearrange("p (d c) -> p d c", d=8))
                nc.gpsimd.collective_compute(
                    "AllToAll", mybir.AluOpType.bypass,
                    replica_groups=[list(range(N_CORES))],
                    ins=[a2_in.opt()], outs=[a2_out.opt()])
                g5_raw = fsb.tile([128, 512], BF, tag="graw")
                nc.sync.dma_start(
                    g5_raw[:, 0:W].rearrange("p (s c) -> p s c", s=8),
                    a2_out.rearrange("s q p xl kl -> (q p) s (xl kl)"))
                g5 = unpermute(g5_raw, XL, KL, "g")  # [y± | xl*48 + kzc]

                # t4: per half, 8 blocks [64,48] -> [48,64]; kz±pad output
                t4 = fsb.tile([128, 512], BF, tag="tstk")
                # zero the pad bands (killed by zero lhsT rows in P6, but
                # uninitialized SBUF could hold NaN patterns)
                nc.vector.memset(t4[KC:N, :], 0)
                nc.vector.memset(t4[N + KC:128, :], 0)
                pt4_re = tps.tile([KC, 8, N], BF, tag="p4re")
                pt4_im = tps.tile([KC, 8, N], BF, tag="p4im")
                pts4 = [pt4_re, pt4_im]
                for half in range(2):
                    src = g5[half * N:(half + 1) * N, :]
                    for t in range(8):
                        blkap = src[:, t * KC:(t + 1) * KC]
                        nc.tensor.transpose(
                            pts4[half][:, t, :], blkap,
                            ident_bf2[half * N:half * N + N, :])
                for half in range(2):
                    dst = t4[half * N:half * N + KC, :].rearrange(
                        "w (t i) -> w t i", t=8, i=N)
                    if half == 0:
                        nc.vector.tensor_copy(dst, pts4[half][:])
                    else:
                        nc.scalar.activation(dst, pts4[half][:], Copy)

                # P6: inverse z (real), two accumulating half matmuls
                ps6 = ps6p.tile([N, 512], FP, tag="ps6")
                nc.tensor.matmul(ps6[:], lhsT=WBre_bf[0:N, :],
                                 rhs=t4[0:N, :], start=True, stop=False)
                nc.tensor.matmul(ps6[:], lhsT=WBre_bf[N:128, :],
                                 rhs=t4[N:128, :], start=False, stop=True)
                out_sb = fsb.tile([N, 512], FP, tag="osb")
                nc.vector.tensor_copy(out_sb[:], ps6[:])
                nc.sync.dma_start(out[:], out_sb[:])
            fsb_pool.__exit__(None, None, None)

    nc.compile()
    return nc


def _get_compiled():
    global _COMPILED
    if _COMPILED is None:
        _COMPILED = _build()
    return _COMPILED


def _host_inputs(X, aw, bw, real_grid_flat, hamming):
    X = np.asarray(X, np.float32)
    aw = np.asarray(aw, np.float32)
    bw = np.asarray(bw, np.float32)
    grid = np.asarray(real_grid_flat, np.float32)
    hamming = np.asarray(hamming, np.float32)

    arr = grid.reshape(N, N, N, 3)
    xs = arr[:, 0, 0, 0]
    ys = arr[0, :, 0, 1]
    zs = arr[0, 0, :, 2]
    d2y = (ys[None, :] - X[:, 1:2]) ** 2
    d2z = (zs[None, :] - X[:, 2:3]) ** 2

    lnaw3 = (np.log(np.maximum(aw, 1e-38)) / 3.0).astype(np.float32)
    bwln = np.concatenate([bw, lnaw3], 1).astype(np.float32)

    F = np.fft.fft(np.eye(N), axis=0, norm='ortho')
    IF = np.fft.ifft(np.eye(N), axis=0, norm='ortho')
    perm = (np.arange(N) + N // 2) % N
    Am = F[:, perm]
    Bm = IF[perm, :]
    ks = list(range(17)) + list(range(48, 64))      # 33 surviving bins
    Ac = np.zeros((KC, N), np.complex128); Ac[:33] = Am[ks, :]
    Bc = np.zeros((N, KC), np.complex128); Bc[:, :33] = Bm[:, ks]
    Arc = Ac.real.astype(np.float32); Aic = Ac.imag.astype(np.float32)
    Brc = Bc.real.astype(np.float32); Bic = Bc.imag.astype(np.float32)

    def padc(m, rows=None):
        # pad cols 48->64 (and optionally rows) with zeros
        r = m.shape[0] if rows is None else rows
        p = np.zeros((r, N), np.float32)
        p[0:m.shape[0], 0:m.shape[1]] = m
        return p

    # lhsT blocks with kz/ky/kx compact indices padded to 64
    ArT = padc(Arc.T)            # [64, 64] (48 live cols)
    AiT = padc(Aic.T)
    BrT = np.zeros((N, N), np.float32); BrT[0:KC, :] = Brc.T   # [64, 64]
    BiT = np.zeros((N, N), np.float32); BiT[0:KC, :] = Bic.T
    WA_T = np.block([[ArT, AiT], [-AiT, ArT]]).astype(np.float32)
    WB_T = np.block([[BrT, BiT], [-BiT, BrT]]).astype(np.float32)
    WBre_T = np.concatenate([BrT, -BiT], 0).astype(np.float32)  # (128, 64)
    Az2T = np.zeros((128, 128), np.float32)
    Az2T[0:N, :] = np.concatenate([ArT, AiT], 1)

    Hfull = np.fft.ifftshift(hamming)
    Hc = np.zeros((KC, KC, KC), np.float32)
    Hc[:33, :33, :33] = Hfull[np.ix_(ks, ks, ks)]

    identp = np.zeros((128, N), np.float32)
    identp[0:N, :] = np.eye(N, dtype=np.float32)
    identp[N:128, :] = np.eye(N, dtype=np.float32)

    in_maps = []
    for c in range(N_CORES):
        d2x = (xs[None, 8 * c:8 * (c + 1)] - X[:, 0:1]) ** 2
        d2all = np.concatenate([d2x, d2y, d2z], 1).astype(np.float32)
        din = np.concatenate([d2all, bwln], 1).astype(np.float32)
        # H2: [kx±pad(128), kl*64 + ky-pad]
        H6 = np.zeros((N, KL, N), np.float32)
        H6[0:KC, :, 0:KC] = np.transpose(
            Hc[:, :, KL * c:KL * (c + 1)], (0, 2, 1))
        Hcc = H6.reshape(N, W)
        H2c = np.concatenate([Hcc, Hcc], 0).astype(np.float32)  # (128, 384)
        cm = np.concatenate(
            [WA_T, WB_T, WBre_T, H2c, identp, Az2T], 1).astype(np.float32)
        in_maps.append({"din": din, "cmats": cm})
    return in_maps


def kernel(X, aw, bw, real_grid_flat, hamming):
    global LAST_EXEC_NS, LAST_RESULTS, LAST_BKR
    in_maps = _host_inputs(X, aw, bw, real_grid_flat, hamming)
    nc = _get_compiled()

    trace = bool(os.environ.get("BASS_TRACE"))
    res = run_bass_kernel_spmd(nc, in_maps, core_ids=list(range(N_CORES)),
                               trace=trace)
    LAST_EXEC_NS = res.exec_time_ns
    LAST_RESULTS = res.results
    LAST_BKR = res

    full = np.empty((N, N, N), np.float32)
    for c in range(N_CORES):
        full[:, 8 * c:8 * (c + 1), :] = res.results[c]["out"].reshape(N, 8, N)
    o = np.transpose(full, (1, 2, 0))
    o = (o - o.mean()) / (o.std() + 1e-8)
    return o.astype(np.float32)
